# revision 20
# baseline (speedup 1.0000x reference)
"""Trainium2 Bass kernel for top-1 MoE expert MLP (nn_Experts problem).

Strategy (expert-parallel, one expert per NeuronCore):
  - Routing is one-hot top-1: each token is processed by exactly one expert,
    so each core computes the MLP only for the tokens routed to its expert.
  - Host-side shard step: compute token->expert assignment from
    dispatch_tensor, gather each expert's tokens (transposed to [D, CAP]),
    cast everything to bf16 (rel err ~4e-3 << the 2e-2 gate; halves HBM
    traffic vs fp32/fp32r and enables fast-weight-load on the PE), and pack
    weights into per-tile column blocks so every DMA is contiguous.
  - Device phase A: h^T[F, CAP] = gelu(w1^T @ x^T + b1)    (moving = tokens)
  - Device phase B: y^T[D, CAP] = (w2^T @ h^T) * gate      (moving = tokens)
    Keeping tokens as the moving dim in BOTH phases makes total PE work
    exactly 2*8*16*CAP cycles with no partial-tile waste (the old phase B
    moved w2 columns, so the 64-row token tail tile cost a full 512-cycle
    pass).  Gate is per-token (a per-COLUMN broadcast in y^T layout), so the
    host ships a [128, CAP] replicated gate tile and the DVE does one
    tensor*tensor multiply per output tile.
  - Host-side unshard: scatter y^T columns back to token order, add b2.

DMA: weights bf16 = 8.4 MB/core total (vs 16.8 fp32), under half of what
phase A+B's ~60us of PE time can absorb at ~358 GB/s.  x/w1/y ride the sync
HWDGE ring, w2 rides the scalar HWDGE ring so the two streams don't
head-of-line block each other.  All tiles get dedicated SBUF slots (no pool
rotation) so no DMA ever stalls on a write-after-read dependency.

PE warm-up: the HAM clock gate keeps the PE at 1.2 GHz until it has been
busy for a ~3.4us activity window.  A memset scratch tile + a short burst of
dummy matmuls issued before the first real matmul (which must wait ~8us for
the head DMA) gets the un-throttle out of the way during time that is
DMA-latency anyway.
"""

import numpy as np

B, N, D, E, F = 8, 512, 1024, 8, 2048
T = B * N
P = 128
CAP = 552            # per-expert token capacity (seed-0 max count is 549)
KT1 = D // P         # 8  k-tiles for matmul1 (contract over D)
MT1 = F // P         # 16 m-tiles for matmul1 / k-tiles for matmul2
DT1 = D // P         # 8  output d-tiles for matmul2
C0 = 296             # token chunk split (PSUM bank = 512 fp32 max)
CHUNKS = ((0, C0), (C0, CAP))
W1_BLOCKS = (1, 2, 4, 4, 4)          # m1 = 1..15 DMA block sizes
W1_OFF = (1, 2, 4, 8, 12)
N_WARM = 6           # dummy matmuls to lift the HAM clock gate pre-head

_NC_CACHE = {}


def _build_bass():
    import concourse.bacc as bacc
    import concourse.tile as tile
    from concourse import mybir

    f32 = mybir.dt.float32
    bf16 = mybir.dt.bfloat16

    nc = bacc.Bacc(None, target_bir_lowering=False)
    # head packs xT k-block 0 together with the w1 m1=0 block so ONE DMA
    # unblocks the very first matmul.
    head = nc.declare_dram_parameter("head", [P, CAP + D], bf16, isOutput=False)
    xr = nc.declare_dram_parameter("xr", [P, KT1 - 1, CAP], bf16, isOutput=False)
    w1r = nc.declare_dram_parameter("w1r", [P, MT1 - 1, D], bf16, isOutput=False)
    w2r = nc.declare_dram_parameter("w2r", [P, MT1, D], bf16, isOutput=False)
    gbg = nc.declare_dram_parameter("gbg", [P, MT1 + CAP], f32, isOutput=False)
    y = nc.declare_dram_parameter("y", [D, CAP], bf16, isOutput=True)

    HA = CAP + P  # head_a: xT k=0 block + w1 (m1=0, k=0) tile

    with tile.TileContext(nc) as tc:
        with (
            tc.tile_pool(name="wp", bufs=1) as wp,
            tc.tile_pool(name="hdp", bufs=1) as hdp,
            tc.tile_pool(name="xp", bufs=1) as xp,
            tc.tile_pool(name="w1p", bufs=1) as w1p,
            tc.tile_pool(name="w2p", bufs=1) as w2p,
            tc.tile_pool(name="gp", bufs=1) as gp,
            tc.tile_pool(name="hp", bufs=MT1) as hp,
            tc.tile_pool(name="stp", bufs=3) as stp,
            tc.tile_pool(name="ps", bufs=4, space="PSUM") as ps,
        ):
            # --- PE warm-up: dummy matmuls on a zeroed scratch tile ---
            # Dummies write into what will become m1=0's psum tile; the real
            # accumulation group's start=True overwrites them (has_written
            # clear), so no dedicated PSUM bank is needed.  One shared
            # 4-deep psum pool serves phase A and B: 4 bufs x 2 tags =
            # all 8 banks, and gives the scalar engine ~7us of slack to
            # drain each activation.
            warm = wp.tile([P, C0], bf16, tag="warm")
            nc.vector.memset(warm[:], 0.0)
            wps = ps.tile([P, C0], f32, tag="psc0", name="warm_ps")
            for i in range(N_WARM):
                nc.tensor.matmul(wps[:], warm[:, 0:P], warm[:],
                                 start=True, stop=True, skip_group_check=True)

            # --- DMA plan.  Two independent FIFO HWDGE rings (sync, scalar)
            # share ~358 GB/s of HBM.  Completion semaphores come from an
            # 8-lane round-robin pool keyed on EMISSION order, so a DMA may
            # have to wait for the transfer 8 emissions earlier to fully
            # complete — keep the big w2 transfers out of the first 8 slots
            # and anchor them in the scalar queue after m1=3 / m1=8
            # activations so they don't steal bandwidth from the phase-A
            # critical path.
            head_a = hdp.tile([P, HA], bf16, tag="ha")
            nc.sync.dma_start(out=head_a[:], in_=head[:, 0:HA])       # 1
            head_b = hdp.tile([P, D - P], bf16, tag="hb")
            nc.scalar.dma_start(out=head_b[:], in_=head[:, HA:])      # 2

            xa = xp.tile([P, 3, CAP], bf16, tag="xa")
            nc.sync.dma_start(out=xa[:], in_=xr[:, 0:3, :])           # 3
            xb = xp.tile([P, 4, CAP], bf16, tag="xb")
            nc.scalar.dma_start(out=xb[:], in_=xr[:, 3:7, :])         # 4

            w1_t = []
            for j, nm in enumerate(W1_BLOCKS):
                w1_t.append(w1p.tile([P, nm, D], bf16, tag=f"w1_{j}",
                                     name=f"w1_{j}"))

            def load_w1(j, eng):
                r0 = W1_OFF[j] - 1
                eng.dma_start(out=w1_t[j][:],
                              in_=w1r[:, r0:r0 + W1_BLOCKS[j], :])

            load_w1(0, nc.sync)                                       # 5
            load_w1(1, nc.scalar)                                     # 6
            load_w1(2, nc.sync)                                       # 7
            load_w1(3, nc.sync)                                       # 8
            load_w1(4, nc.sync)                                       # 9
            gb_sb = gp.tile([P, MT1 + CAP], f32, tag="gbg")
            nc.scalar.dma_start(out=gb_sb[:], in_=gbg[:, :])          # 10

            w2_t = [w2p.tile([P, 8, D], bf16, tag=f"w2_{j}",
                             name=f"w2_{j}") for j in range(2)]
            # The scheduler hoists dependency-free dma_starts to the front
            # of their queue, so "emit late" alone cannot delay the big w2
            # transfers past the phase-A w1 crunch.  Instead each w2 DMA
            # gets a real WAR dependency: a tiny tensor-queue matmul reads
            # a (memset-initialized) corner of the destination tile at the
            # right point in phase A, and the DMA rides the otherwise-empty
            # gpsimd (SWDGE) ring so its wait cannot block anything else.
            for j in range(2):
                nc.vector.memset(w2_t[j][:, 0, 0:256], 0.0)

            def delay_read_w2(j, tgt):
                nc.tensor.matmul(tgt[:2, :256], w2_t[j][:, 0, 0:2],
                                 w2_t[j][:, 0, 0:256],
                                 start=True, stop=True,
                                 skip_group_check=True)

            def load_w2(j):                                           # 11, 12
                nc.gpsimd.dma_start(out=w2_t[j][:],
                                    in_=w2r[:, j * 8:(j + 1) * 8, :])

            def x_mv(k, a, b):
                if k == 0:
                    return head_a[:, a:b]
                if k <= 3:
                    return xa[:, k - 1, a:b]
                return xb[:, k - 4, a:b]

            def w1_lhs(m1, k):
                if m1 == 0:
                    if k == 0:
                        return head_a[:, CAP:CAP + P]
                    return head_b[:, (k - 1) * P:k * P]
                j = next(i for i in range(len(W1_BLOCKS))
                         if W1_OFF[i] <= m1 < W1_OFF[i] + W1_BLOCKS[i])
                return w1_t[j][:, m1 - W1_OFF[j], k * P:(k + 1) * P]

            def primer(lhs1, rhs1, tgt):
                # touches a fresh w1 block on PE so later matmuls stay
                # single-wait; writes into the upcoming group's psum tile,
                # whose start=True overwrites the garbage
                nc.tensor.matmul(tgt[:2, :256], lhs1, rhs1,
                                 start=True, stop=True, skip_group_check=True)

            # --- Phase A: h^T[F, CAP] = gelu(w1^T @ x^T + b1) ---
            gelu = mybir.ActivationFunctionType.Gelu
            h_sb = []
            for m1 in range(MT1):
                if m1 == 0:
                    pss = [wps,
                           ps.tile([P, CAP - C0], f32, tag="psc1",
                                   name="psc1_0")]
                else:
                    pss = [ps.tile([P, b - a], f32, tag=f"psc{i}",
                                   name=f"psc{i}_{m1}")
                           for i, (a, b) in enumerate(CHUNKS)]
                if m1 in W1_OFF:
                    blk = w1_t[W1_OFF.index(m1)]
                    primer(blk[:, 0, 0:2], blk[:, 0, 0:256], pss[0])
                if m1 == 4:
                    delay_read_w2(0, pss[0])
                    load_w2(0)
                elif m1 == 9:
                    delay_read_w2(1, pss[0])
                    load_w2(1)
                for k in range(KT1):
                    lhs = w1_lhs(m1, k)
                    st, sp = (k == 0), (k == KT1 - 1)
                    for i, (a, b) in enumerate(CHUNKS):
                        nc.tensor.matmul(pss[i][:], lhs, x_mv(k, a, b),
                                         start=st, stop=sp)
                h = hp.tile([P, CAP], bf16, tag="h", name=f"h_{m1}")
                bias = gb_sb[:, m1:m1 + 1]
                for i, (a, b) in enumerate(CHUNKS):
                    nc.scalar.activation(h[:, a:b], pss[i][:], gelu, bias=bias)
                h_sb.append(h)

            # --- Phase B: y^T[D, CAP] = (w2^T @ h^T) * gate ---
            def b_mm(pt, d, k2, a, b):
                lhs = w2_t[k2 // 8][:, k2 % 8, d * P:(d + 1) * P]
                nc.tensor.matmul(pt[:], lhs, h_sb[k2][:, a:b],
                                 start=(k2 == 0), stop=(k2 == MT1 - 1))

            for d in range(DT1 - 1):
                pss = [ps.tile([P, b - a], f32, tag=f"psc{i}",
                               name=f"psc{i}_b{d}")
                       for i, (a, b) in enumerate(CHUNKS)]
                for k2 in range(MT1):
                    for i, (a, b) in enumerate(CHUNKS):
                        b_mm(pss[i], d, k2, a, b)
                stage = stp.tile([P, CAP], bf16, tag="stage", name=f"st_{d}")
                for i, (a, b) in enumerate(CHUNKS):
                    nc.vector.tensor_mul(stage[:, a:b], pss[i][:],
                                         gb_sb[:, MT1 + a:MT1 + b])
                nc.sync.dma_start(out=y[d * P:(d + 1) * P, :], in_=stage[:])
            # last tile: the two chunks run as sequential k2 loops so the
            # first chunk's mul+DMA issue ~2us before the final matmul
            d = DT1 - 1
            stage = stp.tile([P, CAP], bf16, tag="stage", name=f"st_{d}")
            for i, (a, b) in ((1, CHUNKS[1]), (0, CHUNKS[0])):
                pt = ps.tile([P, b - a], f32, tag=f"psc{i}",
                             name=f"psc{i}_b{d}")
                for k2 in range(MT1):
                    b_mm(pt, d, k2, a, b)
                nc.vector.tensor_mul(stage[:, a:b], pt[:],
                                     gb_sb[:, MT1 + a:MT1 + b])
                nc.sync.dma_start(out=y[d * P:(d + 1) * P, a:b],
                                  in_=stage[:, a:b])
    if not nc.is_finalized():
        nc.finalize()
    return nc


def _get_nc():
    if "nc" not in _NC_CACHE:
        _NC_CACHE["nc"] = _build_bass()
    return _NC_CACHE["nc"]


def kernel(x, dispatch_tensor, combine_tensor, w1, b1, w2, b2, **_):
    from concourse.bass_utils import run_bass_kernel_spmd
    from concourse import mybir

    bf = mybir.dt.np(mybir.dt.bfloat16)

    x = np.ascontiguousarray(np.asarray(x, dtype=np.float32)).reshape(T, D)
    dispatch = np.asarray(dispatch_tensor, dtype=np.float32).reshape(T, E)
    combine = np.asarray(combine_tensor, dtype=np.float32).reshape(T, E)
    w1 = np.asarray(w1, dtype=np.float32)
    b1 = np.asarray(b1, dtype=np.float32)
    w2 = np.asarray(w2, dtype=np.float32)
    b2 = np.asarray(b2, dtype=np.float32)

    top = dispatch.argmax(-1)
    gate = combine.sum(-1)
    full = [np.nonzero(top == e)[0] for e in range(E)]
    idxs = [idx[:CAP] for idx in full]
    spill = [idx[CAP:] for idx in full]  # never non-empty for T=4096, E=8

    in_maps = []
    for e in range(E):
        idx = idxs[e]
        c = len(idx)
        xT = np.zeros((D, CAP), bf)
        xT[:, :c] = x[idx].T.astype(bf)
        # w1s[m1, p, k*P+m] = w1[k*P+p, m1*P+m]: per-m1 [P, D] blocks whose
        # [:, k*P:(k+1)*P] slice is the lhsT k-tile for output tile m1.
        w1s = np.ascontiguousarray(
            w1[e].reshape(KT1, P, MT1, P).transpose(2, 1, 0, 3)
        ).astype(bf).reshape(MT1, P, D)
        # w2s[k2, p, d] = w2[k2*P+p, d]: lhsT tiles for phase B.
        w2s = np.ascontiguousarray(
            w2[e].reshape(MT1, P, D)).astype(bf).transpose(1, 0, 2)
        gbgv = np.zeros((P, MT1 + CAP), np.float32)
        gbgv[:, :MT1] = b1[e].reshape(MT1, P).T
        g = np.zeros(CAP, np.float32)
        g[:c] = gate[idx]
        gbgv[:, MT1:] = np.broadcast_to(g[None, :], (P, CAP))
        in_maps.append({
            "head": np.ascontiguousarray(
                np.concatenate([xT[:P], w1s[0]], axis=1)),
            "xr": np.ascontiguousarray(
                xT[P:].reshape(KT1 - 1, P, CAP).transpose(1, 0, 2)),
            "w1r": np.ascontiguousarray(w1s[1:].transpose(1, 0, 2)),
            "w2r": np.ascontiguousarray(w2s),
            "gbg": gbgv,
        })

    global _LAST_IN_MAPS
    _LAST_IN_MAPS = in_maps
    nc = _get_nc()
    res = run_bass_kernel_spmd(nc, in_maps, list(range(E)))

    y_flat = np.empty((T, D), np.float32)
    for e in range(E):
        c = len(idxs[e])
        y_flat[idxs[e]] = np.asarray(res.results[e]["y"],
                                     dtype=np.float32).T[:c]
        if len(spill[e]):
            # capacity-overflow fallback (exact fp32 math on host); unused
            # for the reference shapes but keeps any input correct.
            import math

            erf = np.frompyfunc(math.erf, 1, 1)
            hs = x[spill[e]] @ w1[e] + b1[e]
            hs = hs * 0.5 * (1.0 + erf(hs / np.sqrt(2.0)).astype(np.float64))
            y_flat[spill[e]] = (hs @ w2[e]) * gate[spill[e]][:, None]
    return (y_flat + b2[None, :]).reshape(B, N, D)


# revision 28
# speedup vs baseline: 1.1859x; 1.1859x over previous
"""Trainium2 Bass kernel for top-1 MoE expert MLP (nn_Experts problem).

Strategy (expert-parallel, one expert per NeuronCore):
  - Routing is one-hot top-1: each token is processed by exactly one expert,
    so each core computes the MLP only for the tokens routed to its expert.
  - Host-side shard step: compute token->expert assignment from
    dispatch_tensor, gather each expert's tokens (transposed to [D, CAP]),
    cast everything to bf16 (rel err ~4e-3 << the 2e-2 gate; halves HBM
    traffic vs fp32/fp32r and enables fast-weight-load on the PE), and pack
    weights into per-tile column blocks so every DMA is contiguous.
  - Device phase A: h^T[F, CAP] = gelu(w1^T @ x^T + b1)    (moving = tokens)
  - Device phase B: y^T[D, CAP] = (w2^T @ h^T) * gate      (moving = tokens)
    Keeping tokens as the moving dim in BOTH phases makes total PE work
    exactly 2*8*16*CAP cycles with no partial-tile waste (the old phase B
    moved w2 columns, so the 64-row token tail tile cost a full 512-cycle
    pass).  Gate is per-token (a per-COLUMN broadcast in y^T layout), so the
    host ships a [128, CAP] replicated gate tile and the DVE does one
    tensor*tensor multiply per output tile.
  - Host-side unshard: scatter y^T columns back to token order, add b2.

DMA: weights bf16 = 8.4 MB/core total (vs 16.8 fp32), under half of what
phase A+B's ~60us of PE time can absorb at ~358 GB/s.  x/w1/y ride the sync
HWDGE ring, w2 rides the scalar HWDGE ring so the two streams don't
head-of-line block each other.  All tiles get dedicated SBUF slots (no pool
rotation) so no DMA ever stalls on a write-after-read dependency.

PE warm-up: the HAM clock gate keeps the PE at 1.2 GHz until it has been
busy for a ~3.4us activity window.  A memset scratch tile + a short burst of
dummy matmuls issued before the first real matmul (which must wait ~8us for
the head DMA) gets the un-throttle out of the way during time that is
DMA-latency anyway.
"""

import numpy as np

B, N, D, E, F = 8, 512, 1024, 8, 2048
T = B * N
P = 128
CAP = 552            # per-expert token capacity (seed-0 max count is 549)
KT1 = D // P         # 8  k-tiles for matmul1 (contract over D)
MT1 = F // P         # 16 m-tiles for matmul1 / k-tiles for matmul2
DT1 = D // P         # 8  output d-tiles for matmul2
C0 = 296             # token chunk split (PSUM bank = 512 fp32 max)
CHUNKS = ((0, C0), (C0, CAP))
W1_BLOCKS = (1, 2, 4, 4, 4)          # m1 = 1..15 DMA block sizes
W1_OFF = (1, 2, 4, 8, 12)
N_WARM = 6           # dummy matmuls to lift the HAM clock gate pre-head

_NC_CACHE = {}


def _build_bass():
    import concourse.bacc as bacc
    import concourse.tile as tile
    from concourse import mybir

    f32 = mybir.dt.float32
    bf16 = mybir.dt.bfloat16

    nc = bacc.Bacc(None, target_bir_lowering=False)
    # head packs xT k-block 0, the w1 m1=0 block, and the b1 bias columns
    # (bf16) so one early DMA pair covers the whole m1=0 critical path.
    head = nc.declare_dram_parameter("head", [P, CAP + D + MT1], bf16,
                                     isOutput=False)
    xr = nc.declare_dram_parameter("xr", [P, KT1 - 1, CAP], bf16, isOutput=False)
    w1r = nc.declare_dram_parameter("w1r", [P, MT1 - 1, D], bf16, isOutput=False)
    w2r = nc.declare_dram_parameter("w2r", [P, MT1, D], bf16, isOutput=False)
    gbg = nc.declare_dram_parameter("gbg", [P, CAP], f32, isOutput=False)
    y = nc.declare_dram_parameter("y", [D, CAP], bf16, isOutput=True)

    HA = CAP + P  # head_a: xT k=0 block + w1 (m1=0, k=0) tile

    with tile.TileContext(nc) as tc:
        with (
            tc.tile_pool(name="wp", bufs=1) as wp,
            tc.tile_pool(name="hdp", bufs=1) as hdp,
            tc.tile_pool(name="xp", bufs=1) as xp,
            tc.tile_pool(name="w1p", bufs=1) as w1p,
            tc.tile_pool(name="w2p", bufs=1) as w2p,
            tc.tile_pool(name="gp", bufs=1) as gp,
            tc.tile_pool(name="hp", bufs=MT1) as hp,
            tc.tile_pool(name="stp", bufs=3) as stp,
            tc.tile_pool(name="psA", bufs=2, space="PSUM") as psA,
            tc.tile_pool(name="psB", bufs=2, space="PSUM") as psB,
        ):
            # --- PE warm-up: dummy matmuls on a zeroed scratch tile ---
            # (borrows a psB slot — idle until phase B).  NOTE: keep each
            # phase's chunk pair inside one PSUM half: a merged 4-deep pool
            # that put c0 in banks 0-3 and c1 in banks 4-7 made EVERY
            # back-to-back matmul/psum-read cross the half boundary and cost
            # +20% on the whole kernel.
            warm = wp.tile([P, C0], bf16, tag="warm")
            nc.vector.memset(warm[:], 0.0)
            wps = psB.tile([P, C0], f32, tag="psB0", name="warm_ps")
            for i in range(N_WARM):
                nc.tensor.matmul(wps[:], warm[:, 0:P], warm[:],
                                 start=True, stop=True, skip_group_check=True)

            # --- DMA plan.  Two independent FIFO HWDGE rings (sync, scalar)
            # share ~358 GB/s of HBM.  Completion semaphores come from an
            # 8-lane round-robin pool keyed on EMISSION order, so a DMA may
            # have to wait for the transfer 8 emissions earlier to fully
            # complete — keep the big w2 transfers out of the first 8 slots
            # and anchor them in the scalar queue after m1=3 / m1=8
            # activations so they don't steal bandwidth from the phase-A
            # critical path.
            head_a = hdp.tile([P, HA], bf16, tag="ha")
            nc.sync.dma_start(out=head_a[:], in_=head[:, 0:HA])       # 1
            head_b = hdp.tile([P, D - P + MT1], bf16, tag="hb")
            nc.scalar.dma_start(out=head_b[:], in_=head[:, HA:])      # 2

            xa = xp.tile([P, 3, CAP], bf16, tag="xa")
            nc.sync.dma_start(out=xa[:], in_=xr[:, 0:3, :])           # 3
            xb = xp.tile([P, 4, CAP], bf16, tag="xb")
            nc.scalar.dma_start(out=xb[:], in_=xr[:, 3:7, :])         # 4

            w1_t = []
            for j, nm in enumerate(W1_BLOCKS):
                w1_t.append(w1p.tile([P, nm, D], bf16, tag=f"w1_{j}",
                                     name=f"w1_{j}"))

            def load_w1(j, eng):
                r0 = W1_OFF[j] - 1
                eng.dma_start(out=w1_t[j][:],
                              in_=w1r[:, r0:r0 + W1_BLOCKS[j], :])

            load_w1(0, nc.sync)                                       # 5
            load_w1(1, nc.scalar)                                     # 6
            load_w1(2, nc.sync)                                       # 7
            load_w1(3, nc.sync)                                       # 8
            load_w1(4, nc.sync)                                       # 9
            gb_sb = gp.tile([P, CAP], f32, tag="gbg")
            nc.scalar.dma_start(out=gb_sb[:], in_=gbg[:, :])          # 10

            w2_t = [w2p.tile([P, 8, D], bf16, tag=f"w2_{j}",
                             name=f"w2_{j}") for j in range(2)]
            # The scheduler hoists dependency-free dma_starts to the front
            # of their queue, so "emit late" alone cannot delay the big w2
            # transfers past the phase-A w1 crunch.  Instead each w2 DMA
            # gets a real WAR dependency: a tiny tensor-queue matmul reads
            # a (memset-initialized) corner of the destination tile at the
            # right point in phase A, and the DMA rides the otherwise-empty
            # gpsimd (SWDGE) ring so its wait cannot block anything else.
            for j in range(2):
                nc.vector.memset(w2_t[j][:, 0, 0:256], 0.0)

            def delay_read_w2(j):
                nc.tensor.matmul(wps[:2, :256], w2_t[j][:, 0, 0:2],
                                 w2_t[j][:, 0, 0:256],
                                 start=True, stop=True,
                                 skip_group_check=True)

            def load_w2(j):                                           # 11, 12
                nc.gpsimd.dma_start(out=w2_t[j][:],
                                    in_=w2r[:, j * 8:(j + 1) * 8, :])

            def x_mv(k, a, b):
                if k == 0:
                    return head_a[:, a:b]
                if k <= 3:
                    return xa[:, k - 1, a:b]
                return xb[:, k - 4, a:b]

            def w1_lhs(m1, k):
                if m1 == 0:
                    if k == 0:
                        return head_a[:, CAP:CAP + P]
                    return head_b[:, (k - 1) * P:k * P]
                j = next(i for i in range(len(W1_BLOCKS))
                         if W1_OFF[i] <= m1 < W1_OFF[i] + W1_BLOCKS[i])
                return w1_t[j][:, m1 - W1_OFF[j], k * P:(k + 1) * P]

            def primer(lhs1, rhs1):
                # touches a fresh w1 block on PE (in the idle warm psum
                # slot) so later matmuls stay single-wait
                nc.tensor.matmul(wps[:2, :256], lhs1, rhs1,
                                 start=True, stop=True, skip_group_check=True)

            # --- Phase A: h^T[F, CAP] = gelu(w1^T @ x^T + b1) ---
            gelu = mybir.ActivationFunctionType.Gelu
            h_sb = []
            for m1 in range(MT1):
                pss = [psA.tile([P, b - a], f32, tag=f"psA{i}",
                                name=f"psA{i}_{m1}")
                       for i, (a, b) in enumerate(CHUNKS)]
                if m1 in W1_OFF:
                    blk = w1_t[W1_OFF.index(m1)]
                    primer(blk[:, 0, 0:2], blk[:, 0, 0:256])
                if m1 == 4:
                    delay_read_w2(0)
                    load_w2(0)
                elif m1 == 9:
                    delay_read_w2(1)
                    load_w2(1)
                for k in range(KT1):
                    lhs = w1_lhs(m1, k)
                    st, sp = (k == 0), (k == KT1 - 1)
                    for i, (a, b) in enumerate(CHUNKS):
                        nc.tensor.matmul(pss[i][:], lhs, x_mv(k, a, b),
                                         start=st, stop=sp)
                h = hp.tile([P, CAP], bf16, tag="h", name=f"h_{m1}")
                bias = head_b[:, D - P + m1:D - P + m1 + 1]
                for i, (a, b) in enumerate(CHUNKS):
                    nc.scalar.activation(h[:, a:b], pss[i][:], gelu, bias=bias)
                h_sb.append(h)

            # --- Phase B: y^T[D, CAP] = (w2^T @ h^T) * gate ---
            def b_mm(pt, d, k2, a, b):
                lhs = w2_t[k2 // 8][:, k2 % 8, d * P:(d + 1) * P]
                nc.tensor.matmul(pt[:], lhs, h_sb[k2][:, a:b],
                                 start=(k2 == 0), stop=(k2 == MT1 - 1))

            for d in range(DT1 - 1):
                pss = [psB.tile([P, b - a], f32, tag=f"psB{i}",
                                name=f"psB{i}_b{d}")
                       for i, (a, b) in enumerate(CHUNKS)]
                for k2 in range(MT1):
                    for i, (a, b) in enumerate(CHUNKS):
                        b_mm(pss[i], d, k2, a, b)
                stage = stp.tile([P, CAP], bf16, tag="stage", name=f"st_{d}")
                for i, (a, b) in enumerate(CHUNKS):
                    nc.vector.tensor_mul(stage[:, a:b], pss[i][:],
                                         gb_sb[:, a:b])
                nc.sync.dma_start(out=y[d * P:(d + 1) * P, :], in_=stage[:])
            # last tile: the two chunks run as sequential k2 loops so the
            # first chunk's mul+DMA issue ~2us before the final matmul
            d = DT1 - 1
            stage = stp.tile([P, CAP], bf16, tag="stage", name=f"st_{d}")
            for i, (a, b) in ((1, CHUNKS[1]), (0, CHUNKS[0])):
                pt = psB.tile([P, b - a], f32, tag=f"psB{i}",
                              name=f"psB{i}_b{d}")
                for k2 in range(MT1):
                    b_mm(pt, d, k2, a, b)
                nc.vector.tensor_mul(stage[:, a:b], pt[:],
                                     gb_sb[:, a:b])
                nc.sync.dma_start(out=y[d * P:(d + 1) * P, a:b],
                                  in_=stage[:, a:b])
    if not nc.is_finalized():
        nc.finalize()
    return nc


def _get_nc():
    if "nc" not in _NC_CACHE:
        _NC_CACHE["nc"] = _build_bass()
    return _NC_CACHE["nc"]


def kernel(x, dispatch_tensor, combine_tensor, w1, b1, w2, b2, **_):
    from concourse.bass_utils import run_bass_kernel_spmd
    from concourse import mybir

    bf = mybir.dt.np(mybir.dt.bfloat16)

    x = np.ascontiguousarray(np.asarray(x, dtype=np.float32)).reshape(T, D)
    dispatch = np.asarray(dispatch_tensor, dtype=np.float32).reshape(T, E)
    combine = np.asarray(combine_tensor, dtype=np.float32).reshape(T, E)
    w1 = np.asarray(w1, dtype=np.float32)
    b1 = np.asarray(b1, dtype=np.float32)
    w2 = np.asarray(w2, dtype=np.float32)
    b2 = np.asarray(b2, dtype=np.float32)

    top = dispatch.argmax(-1)
    gate = combine.sum(-1)
    full = [np.nonzero(top == e)[0] for e in range(E)]
    idxs = [idx[:CAP] for idx in full]
    spill = [idx[CAP:] for idx in full]  # never non-empty for T=4096, E=8

    in_maps = []
    for e in range(E):
        idx = idxs[e]
        c = len(idx)
        xT = np.zeros((D, CAP), bf)
        xT[:, :c] = x[idx].T.astype(bf)
        # w1s[m1, p, k*P+m] = w1[k*P+p, m1*P+m]: per-m1 [P, D] blocks whose
        # [:, k*P:(k+1)*P] slice is the lhsT k-tile for output tile m1.
        w1s = np.ascontiguousarray(
            w1[e].reshape(KT1, P, MT1, P).transpose(2, 1, 0, 3)
        ).astype(bf).reshape(MT1, P, D)
        # w2s[k2, p, d] = w2[k2*P+p, d]: lhsT tiles for phase B.
        w2s = np.ascontiguousarray(
            w2[e].reshape(MT1, P, D)).astype(bf).transpose(1, 0, 2)
        g = np.zeros(CAP, np.float32)
        g[:c] = gate[idx]
        gbgv = np.ascontiguousarray(
            np.broadcast_to(g[None, :], (P, CAP)))
        b1t = b1[e].reshape(MT1, P).T.astype(bf)  # [P, MT1] bias columns
        in_maps.append({
            "head": np.ascontiguousarray(
                np.concatenate([xT[:P], w1s[0], b1t], axis=1)),
            "xr": np.ascontiguousarray(
                xT[P:].reshape(KT1 - 1, P, CAP).transpose(1, 0, 2)),
            "w1r": np.ascontiguousarray(w1s[1:].transpose(1, 0, 2)),
            "w2r": np.ascontiguousarray(w2s),
            "gbg": gbgv,
        })

    global _LAST_IN_MAPS
    _LAST_IN_MAPS = in_maps
    nc = _get_nc()
    res = run_bass_kernel_spmd(nc, in_maps, list(range(E)))

    y_flat = np.empty((T, D), np.float32)
    for e in range(E):
        c = len(idxs[e])
        y_flat[idxs[e]] = np.asarray(res.results[e]["y"],
                                     dtype=np.float32).T[:c]
        if len(spill[e]):
            # capacity-overflow fallback (exact fp32 math on host); unused
            # for the reference shapes but keeps any input correct.
            import math

            erf = np.frompyfunc(math.erf, 1, 1)
            hs = x[spill[e]] @ w1[e] + b1[e]
            hs = hs * 0.5 * (1.0 + erf(hs / np.sqrt(2.0)).astype(np.float64))
            y_flat[spill[e]] = (hs @ w2[e]) * gate[spill[e]][:, None]
    return (y_flat + b2[None, :]).reshape(B, N, D)
